# revision 9
# baseline (speedup 1.0000x reference)
"""Mean-shift clustering (3 iterations) on 8 TRN2 NeuronCores.

Column-parallel sharding: core c owns columns [c*NL, (c+1)*NL) of X [D, N].
Each iteration, per core:
  S_blk[i, jl] = sum_d X[d, i] * Xl[d, jl]          (PE, i in 64 chunks of 128)
  K_blk = exp(6 * S_blk)                            (ScalarE, PSUM -> SBUF)
  XKT[jl, f] = sum_i K[i, jl] * XtA[i, f]           (PE, accumulated over chunks)
      where XtA = [X^T | ones] is [N, D+1]; column D gives colsum(K) free.
  out_T[jl, :] = XKT[jl, 0:D] / XKT[jl, D]          (reciprocal + tensor_scalar)
  out_f = out_T^T (PE transposes) -> next Xl and the per-core output block.
Between iterations an AllGather shares a packed blob per rank:
  [out_f (D*NL) | out_T with ones col (NL*(D+1))], so every core rebuilds the
full X in both layouts with no rank-dependent addressing.
"""

import os
import numpy as np

import concourse.bass as bass
import concourse.mybir as mybir
import concourse.tile as tile
from concourse import bacc
from concourse.bass_utils import run_bass_kernel_spmd
from concourse.masks import make_identity

N_CORES = 8
D = 256            # feature dim
N = 8192           # number of points
NL = N // N_CORES  # 1024 columns per core
DELTA = 6.0
ITERS = 3
NCH = N // 128     # 64 i-chunks
W_PER_CB = NL // 128   # 8 i-chunks per rank block
DP1 = D + 1        # 257: ones column index is D
DPAD = 260         # i-major tile width, padded to a multiple of 4 for fp32r
F_BLOB = D * NL            # 262144 floats, f-major block
T_BLOB = NL * DPAD         # i-major block (with ones + zero pad)
BLOB = F_BLOB + T_BLOB     # 525312 floats per rank
JL_HALF = NL // 2          # 512, one pass of jl

_MM_DT_NAME = os.environ.get("MS_MM_DT", "float32r")
F32 = mybir.dt.float32
# Matmul streaming dtype: float32r streams at 1 cyc/row (vs 4 for float32) when
# the free dim is >= 256. The BIR verifier requires every producer of an fp32r
# matmul input to emit fp32r, so the whole X/K tile chain is typed fp32r.
MDT = mybir.dt.float32r if _MM_DT_NAME == "float32r" else F32


def _mm(ap):
    return ap


def _blob_f_view(src, cb, dc):
    """[128, NL] view: rows dc*128..+128 of rank cb's f-major block."""
    off = dc * 128 * NL
    return src[cb : cb + 1, off : off + 128 * NL].rearrange(
        "o (p f) -> (o p) f", p=128
    )


def _blob_t_view(src, m):
    """[128, DP1] view: i-major chunk m (global rows m*128..+128 of [N, DP1])."""
    cb, w = divmod(m, W_PER_CB)
    off = F_BLOB + w * 128 * DPAD
    return src[cb : cb + 1, off : off + 128 * DPAD].rearrange(
        "o (p f) -> (o p) f", p=128
    )


def _agin_f_view(ag_in, dc):
    off = dc * 128 * NL
    return ag_in[0:1, off : off + 128 * NL].rearrange("o (p f) -> (o p) f", p=128)


def _agin_t_view(ag_in, j):
    off = F_BLOB + j * 128 * DPAD
    return ag_in[0:1, off : off + 128 * DPAD].rearrange("o (p f) -> (o p) f", p=128)


def build_module():
    nc = bacc.Bacc("TRN2", target_bir_lowering=False, debug=False,
                   num_devices=N_CORES)
    blob0 = nc.dram_tensor("BLOB0", [1, BLOB], MDT, kind="ExternalInput").ap()
    xl_in = nc.dram_tensor("XL", [D, NL], MDT, kind="ExternalInput").ap()
    outs = [
        nc.dram_tensor(f"OUT{t}", [D, NL], F32, kind="ExternalOutput").ap()
        for t in range(ITERS)
    ]

    with tile.TileContext(nc) as tc:
        with (
            tc.tile_pool(name="const", bufs=1) as constp,
            tc.tile_pool(name="xf", bufs=1) as xfp,
            tc.tile_pool(name="xt", bufs=1) as xtp,
            tc.tile_pool(name="xl", bufs=1) as xlp,
            tc.tile_pool(name="k", bufs=3) as kp,
            tc.tile_pool(name="ot", bufs=1) as otp,
            tc.tile_pool(name="rc", bufs=1) as rcp,
            tc.tile_pool(name="spsum", bufs=2, space="PSUM") as spp,
            tc.tile_pool(name="xkt", bufs=1, space="PSUM") as xktp,
            tc.tile_pool(name="trp", bufs=2, space="PSUM") as trp,
            tc.tile_pool(name="dram", bufs=4, space="DRAM") as dram,
        ):
            ident = constp.tile([128, 128], F32, name="ident")
            make_identity(nc, ident[:])

            # Persistent SBUF tiles, rewritten each iteration.
            xf = [[xfp.tile([128, NL], MDT, name=f"xf_{dc}_{cb}", tag=f"xf_{dc}_{cb}")
                   for cb in range(N_CORES)] for dc in range(2)]
            xt = [xtp.tile([128, DPAD], MDT, name=f"xt_{m}", tag=f"xt_{m}")
                  for m in range(NCH)]
            xl_a = [xlp.tile([128, NL], MDT, name=f"xla_{dc}", tag=f"xla_{dc}")
                    for dc in range(2)]
            xl_b = [xlp.tile([128, NL], MDT, name=f"xlb_{dc}", tag=f"xlb_{dc}")
                    for dc in range(2)]

            ag_ins = [dram.tile([1, BLOB], MDT, name=f"ag_in_{t}")
                      for t in range(ITERS - 1)]
            ag_outs = [dram.tile([N_CORES, BLOB], MDT, name=f"ag_out_{t}")
                       for t in range(ITERS - 1)]
            # iteration-0 gather: each core contributes its own host-built blob
            ag_in0 = dram.tile([1, BLOB], MDT, name="ag_in0")
            ag_out0 = dram.tile([N_CORES, BLOB], MDT, name="ag_out0")
            nc.sync.dma_start(ag_in0[:], blob0[:])
            nc.gpsimd.collective_compute(
                "AllGather",
                mybir.AluOpType.bypass,
                replica_groups=[list(range(N_CORES))],
                ins=[ag_in0.opt()],
                outs=[ag_out0.opt()],
            )

            # iteration 0 consumes the host-prepared replicated blob
            for dc in range(2):
                nc.sync.dma_start(xl_a[dc][:], xl_in[dc * 128 : (dc + 1) * 128, :])

            xl_cur, xl_next = xl_a, xl_b
            for t in range(ITERS):
                src = ag_out0 if t == 0 else ag_outs[t - 1]
                for cb in range(N_CORES):
                    for dc in range(2):
                        nc.sync.dma_start(xf[dc][cb][:], _blob_f_view(src, cb, dc))
                for m in range(NCH):
                    nc.sync.dma_start(xt[m][:], _blob_t_view(src, m))

                out_t_tiles = []
                for p in range(2):  # jl half-passes (PSUM bank budget)
                    acc = [xktp.tile([128, DPAD], F32, name=f"xkt_{t}_{p}_{q}", tag=f"acc{q}")
                           for q in range(4)]
                    for m in range(NCH):
                        cb, w = divmod(m, W_PER_CB)
                        sp = spp.tile([128, JL_HALF], F32, name=f"s_{t}_{p}_{m}", tag="s")
                        for dc in range(2):
                            nc.tensor.matmul(
                                sp[:],
                                _mm(xf[dc][cb][:, w * 128 : (w + 1) * 128]),
                                _mm(xl_cur[dc][:, p * JL_HALF : (p + 1) * JL_HALF]),
                                start=(dc == 0),
                                stop=(dc == 1),
                            )
                        kt = kp.tile([128, JL_HALF], MDT, name=f"k_{t}_{p}_{m}", tag="k")
                        nc.scalar.activation(
                            kt[:], sp[:], mybir.ActivationFunctionType.Exp,
                            scale=DELTA,
                        )
                        for q in range(4):
                            nc.tensor.matmul(
                                acc[q][:],
                                _mm(kt[:, q * 128 : (q + 1) * 128]),
                                _mm(xt[m][:]),
                                start=(m == 0),
                                stop=(m == NCH - 1),
                            )
                    for q in range(4):
                        j = p * 4 + q
                        rc = rcp.tile([128, 1], F32, name=f"rc_{t}_{j}", tag=f"rc{j}")
                        nc.vector.reciprocal(rc[:], acc[q][:, D : D + 1])
                        ot = otp.tile([128, DPAD], F32, name=f"ot_{t}_{j}", tag=f"ot{j}")
                        nc.vector.tensor_scalar_mul(ot[:, 0:D], acc[q][:, 0:D], rc[:])
                        nc.vector.memset(ot[:, D : D + 1], 1.0)
                        nc.vector.memset(ot[:, D + 1 : DPAD], 0.0)
                        out_t_tiles.append(ot)

                # rebuild f-major local block (next Xl / this iteration's output)
                for j in range(8):
                    for dc in range(2):
                        tp = trp.tile([128, 128], F32, name=f"tr_{t}_{j}_{dc}", tag="tr")
                        nc.tensor.transpose(
                            tp[:], out_t_tiles[j][:, dc * 128 : (dc + 1) * 128],
                            ident[:],
                        )
                        nc.vector.tensor_copy(
                            xl_next[dc][:, j * 128 : (j + 1) * 128], tp[:]
                        )
                for dc in range(2):
                    nc.sync.dma_start(
                        outs[t][dc * 128 : (dc + 1) * 128, :],
                        xl_next[dc][:].bitcast(F32),
                    )
                if t < ITERS - 1:
                    ag_in = ag_ins[t]
                    for dc in range(2):
                        nc.sync.dma_start(_agin_f_view(ag_in, dc), xl_next[dc][:])
                    for j in range(8):
                        nc.sync.dma_start(_agin_t_view(ag_in, j).bitcast(F32), out_t_tiles[j][:])
                    nc.gpsimd.collective_compute(
                        "AllGather",
                        mybir.AluOpType.bypass,
                        replica_groups=[list(range(N_CORES))],
                        ins=[ag_ins[t].opt()],
                        outs=[ag_outs[t].opt()],
                    )
                xl_cur, xl_next = xl_next, xl_cur
    nc.compile()
    return nc


_NC_CACHE = None


def _get_module():
    global _NC_CACHE
    if _NC_CACHE is None:
        _NC_CACHE = build_module()
    return _NC_CACHE


def _make_in_maps(X):
    ones = np.ones((NL, 1), dtype=np.float32)
    pad = np.zeros((NL, DPAD - DP1), dtype=np.float32)
    in_maps = []
    for c in range(N_CORES):
        blk = X[:, c * NL : (c + 1) * NL]
        blob = np.empty((1, BLOB), dtype=np.float32)
        blob[0, :F_BLOB] = blk.reshape(-1)
        blob[0, F_BLOB:] = np.concatenate([blk.T, ones, pad], axis=1).reshape(-1)
        in_maps.append({"BLOB0": blob, "XL": np.ascontiguousarray(blk)})
    return in_maps


def run_on_hw(X, trace=False, **kwargs):
    """Run the SPMD kernel; returns (outputs_tuple, BassKernelResults)."""
    nc = _get_module()
    res = run_bass_kernel_spmd(
        nc, _make_in_maps(X), core_ids=list(range(N_CORES)), trace=trace, **kwargs
    )
    outs = tuple(
        np.concatenate([res.results[c][f"OUT{t}"] for c in range(N_CORES)], axis=1)
        for t in range(ITERS)
    )
    return outs, res


def kernel(X):
    X = np.asarray(X, dtype=np.float32)
    assert X.shape == (D, N), X.shape
    outs, _ = run_on_hw(X)
    return outs
